# revision 16
# baseline (speedup 1.0000x reference)
"""GCNII block (knn-9 graph message passing + linear + BN + relu) on 8 TRN2 cores.

Problem (hardcoded): x, x_0: [16, 128, 48, 48] f32; W_lin [128,128]; b_lin,
gamma, beta [128].  N = 48*48 = 2304 tokens per batch, C = 128 channels.

Sharding: data-parallel over batch B (2 batches per core); BN batch stats
all-reduced across the 8 cores.

v5 architecture (single-Gram + DMA-xbar mask transpose):
  Tokens split hi/lo fp16: X ~= Xh + Xl (22-bit effective).  Per 128-row
  block i (phase A): V = Gram(3 fp16 passes: hh, hl, lh) + rank-1(-0.5 sq as
  fp16 hi/lo pair, K=1 each) in PSUM; stash V to SBUF fp32 (ACT copy,
  releases PSUM); per-row V9/V10 via segmented max8 (9 x 256, reading the
  stash) + match_replace; threshold bias tneg = -0.5*(V9+V10) [C,1] fp32;
  mask M_i = Sign(V + tneg) in {-1,+1} fp16 (ACT, per-partition bias,
  graceful on ties), software-pipelined one block behind to keep the scalar
  queue bubble-free -> DMA to DRAM.
  Phase B per column-block j: MT_j = DMA xbar transpose of Mdram[:, j*128:
  (j+1)*128]; NS[c, n] += XTb-block @ MT_j (PE fp16).  No Gram recompute.
  h2 = 0.025*NS + (0.05*X0 + 0.025*total) (DVE fused, total = rowsum Xh via
  ACT accum); out_tok = h2 + W@h2 + 0.5*b (PE fp16); BN partial stats (ACT
  accum); AllReduce stats; y = relu(bn(out_tok) + x + shift via ACT bias).

Exactness: threshold and mask read identical fp32 V values (the stash), so
exactly the top-9 rows are selected wherever V9 != V10 bitwise; a bitwise
tie degrades gracefully (Sign(0)=0 averages the tied pair).  Segmented max8
needs no 256-segment to hold >8 of a row's top-10 (host-verified slack:
worst case 7).
"""

import sys
import types

import numpy as np

# Register the NTFF profile hook if the middleware didn't inject it, so
# BASS_TRACE=1 can capture HW exec time.
try:
    import antenv.axon_hooks  # noqa: F401
except ImportError:
    try:
        from trn_agent_boot.trn_boot import _ntff_profile_via_ctypes

        _mod = types.ModuleType("antenv.axon_hooks")
        _hook = _ntff_profile_via_ctypes("/opt/axon/libaxon_pjrt.so")
        _mod.get_axon_ntff_profile_hook = lambda: _hook
        sys.modules["antenv.axon_hooks"] = _mod
    except Exception:
        pass

import concourse.bass as bass  # noqa: E402
import concourse.tile as tile  # noqa: E402
from concourse import bacc, mybir  # noqa: E402
from concourse.bass_utils import run_bass_kernel_spmd  # noqa: E402

F32 = mybir.dt.float32
FP16 = mybir.dt.float16
AF = mybir.ActivationFunctionType
ALU = mybir.AluOpType

N_CORES = 8
B, C, H, W = 16, 128, 48, 48
N = H * W                      # 2304
BPC = B // N_CORES             # 2 batches per core
NB = N // 128                  # 18 blocks
CHUNKS = [(0, 512), (512, 512), (1024, 512), (1536, 512), (2048, 256)]
SEG = 256
EPS = 1e-5
CNT = float(B * N)

_cache = {}


def _build():
    nc = bacc.Bacc("TRN2", target_bir_lowering=False, debug=False,
                   num_devices=N_CORES)

    x_d = nc.dram_tensor("x", [BPC, C, H, W], F32, kind="ExternalInput")
    x0_d = nc.dram_tensor("x0", [BPC, C, H, W], F32, kind="ExternalInput")
    wT_d = nc.dram_tensor("wT", [C, C], F32, kind="ExternalInput")
    brow_d = nc.dram_tensor("brow", [1, C], F32, kind="ExternalInput")
    gcol_d = nc.dram_tensor("gcol", [C, 1], F32, kind="ExternalInput")
    bcol_d = nc.dram_tensor("bcol", [C, 1], F32, kind="ExternalInput")
    eye_d = nc.dram_tensor("eye", [C, C], F32, kind="ExternalInput")
    out_d = nc.dram_tensor("out", [BPC, C, H, W], F32, kind="ExternalOutput")

    with tile.TileContext(nc) as tc:
        with (
            tc.tile_pool(name="const", bufs=1) as cpool,
            tc.tile_pool(name="work", bufs=1) as wpool,
            tc.tile_pool(name="keep", bufs=1) as kpool,
            tc.tile_pool(name="mask", bufs=3) as mpool,
            tc.tile_pool(name="small", bufs=3) as spool,
            tc.tile_pool(name="chps", bufs=3, space="PSUM") as chpool,
            tc.tile_pool(name="nsps", bufs=1, space="PSUM") as npool,
            tc.tile_pool(name="dram", bufs=1, space="DRAM") as dpool,
        ):
            # ---------------- constants ----------------
            wT_sb = cpool.tile([C, C], F32)
            nc.sync.dma_start(wT_sb[:], wT_d[:])
            eye_sb = cpool.tile([C, C], F32)
            nc.sync.dma_start(eye_sb[:], eye_d[:])
            brow = cpool.tile([1, C], F32)
            nc.sync.dma_start(brow[:], brow_d[:])
            halfb16 = cpool.tile([1, C], FP16)
            nc.vector.tensor_scalar_mul(halfb16[:], brow[:], 0.5)
            gcol = cpool.tile([C, 1], F32)
            nc.sync.dma_start(gcol[:], gcol_d[:])
            bcol = cpool.tile([C, 1], F32)
            nc.sync.dma_start(bcol[:], bcol_d[:])
            ones_r16 = cpool.tile([1, 512], FP16)
            nc.vector.memset(ones_r16[:], 1.0)
            ones_c = cpool.tile([C, 1], F32)
            nc.vector.memset(ones_c[:], 1.0)
            ones16 = cpool.tile([C, C], FP16)
            nc.vector.memset(ones16[:], 1.0)
            wT16 = cpool.tile([C, C], FP16)
            nc.vector.tensor_copy(wT16[:], wT_sb[:])
            eye16 = cpool.tile([C, C], FP16)
            nc.vector.tensor_copy(eye16[:], eye_sb[:])
            s1all = cpool.tile([C, BPC * 5], F32)
            s2all = cpool.tile([C, BPC * 5], F32)

            S = [dict() for _ in range(BPC)]

            # ---------------- prep (both batches) ----------------
            for b in range(BPC):
                st = S[b]
                # prep-transient X (fp32); shares the V32 ring, which is
                # recycled during phase A and reloaded for the finalize
                Xin = wpool.tile([C, N], F32, tag="V32", bufs=2,
                                 name=f"Xin{b}")
                nc.sync.dma_start(Xin[:], x_d[b].rearrange("c h w -> c (h w)"))
                st["Xin"] = Xin
                X0 = wpool.tile([C, N], F32, tag="X0", bufs=2, name=f"X0_{b}")
                nc.scalar.dma_start(X0[:],
                                    x0_d[b].rearrange("c h w -> c (h w)"))
                st["X0"] = X0
            for b in range(BPC):
                st = S[b]
                Xin, X0 = st["Xin"], st["X0"]

                # hi/lo fp16 split of X; total[c] = sum_m Xh[c, m] via accum
                Xh = kpool.tile([C, N], FP16, tag="Xh", bufs=BPC,
                                name=f"Xh{b}")
                totacc = cpool.tile([C, 1], F32, name=f"tot{b}")
                nc.scalar.activation(Xh[:], Xin[:], AF.Copy, accum_out=totacc)
                st["Xh"] = Xh
                Xl = kpool.tile([C, N], FP16, tag="Xl", bufs=BPC,
                                name=f"Xl{b}")
                nc.vector.tensor_sub(Xl[:], Xin[:], Xh[:])
                st["Xl"] = Xl
                # tot05 = 0.5 * total (column, for the h2 fold)
                tot05 = cpool.tile([C, 1], F32, name=f"tot05_{b}")
                nc.vector.tensor_scalar_mul(tot05[:], totacc[:], 0.5)
                st["tot05"] = tot05

                # XTb: block-transposed X (fp16) for NS stationary
                XTb = wpool.tile([C, N], FP16, tag="XTb", bufs=2,
                                 name=f"XT{b}")
                for j in range(NB):
                    pt = chpool.tile([C, C], F32, tag="ch", name="pt")
                    nc.tensor.transpose(pt[:], Xin[:, j * 128:(j + 1) * 128],
                                        eye_sb[:])
                    nc.scalar.copy(XTb[:, j * 128:(j + 1) * 128], pt[:])
                st["XTb"] = XTb

                # sqnr = -0.5 * colsum(X^2) (fp32), then hi/lo fp16 split
                Xsq = wpool.tile([C, N], F32, tag="Xsq", bufs=1, name=f"Xq{b}")
                nc.scalar.square(Xsq[:], Xin[:])
                sqnr = wpool.tile([1, N], F32, tag="sqnr", bufs=1,
                                  name=f"sq{b}")
                for (c0, csz) in CHUNKS:
                    ps = chpool.tile([1, csz], F32, tag="ch", name="sqps")
                    nc.tensor.matmul(ps[:], ones_c[:], Xsq[:, c0:c0 + csz],
                                     start=True, stop=True)
                    nc.vector.tensor_scalar_mul(sqnr[0:1, c0:c0 + csz],
                                                ps[:], -0.5)
                sqh = wpool.tile([C, N], FP16, tag="sqh", bufs=2,
                                 name=f"sqh{b}")
                nc.vector.tensor_copy(sqh[0:1, :], sqnr[0:1, :])
                sql = wpool.tile([C, N], FP16, tag="sql", bufs=2,
                                 name=f"sql{b}")
                nc.vector.tensor_sub(sql[0:1, :], sqnr[0:1, :], sqh[0:1, :])
                for p in (32, 64, 96):
                    nc.sync.dma_start(sqh[p:p + 1, :], sqh[0:1, :])
                    nc.sync.dma_start(sql[p:p + 1, :], sql[0:1, :])
                st["sqh"], st["sql"] = sqh, sql

                # X0t = 0.05*X0 + 0.025*total   (in place over X0)
                nc.vector.tensor_scalar(X0[:], X0[:], tot05[:, 0:1], 0.05,
                                        op0=ALU.add, op1=ALU.mult)

                st["Mdram"] = dpool.tile([N, N], FP16, tag="M", bufs=2,
                                         name=f"M{b}")

            # ---------------- phase A: thresholds + masks ----------------
            pending = []  # (b, i0, V32, tneg) masks delayed by one block

            def flush_mask():
                if not pending:
                    return
                b, i0, V32, tneg = pending.pop()
                M = mpool.tile([C, N], FP16, tag="mT", bufs=2)
                for k, (c0, csz) in enumerate(CHUNKS):
                    nc.scalar.activation(M[:, c0:c0 + csz],
                                         V32[:, c0:c0 + csz], AF.Sign,
                                         bias=tneg[:, 0:1])
                nc.sync.dma_start(S[b]["Mdram"][i0:i0 + 128, :], M[:])

            def phase_a_block(b, i):
                st = S[b]
                Xh, Xl = st["Xh"], st["Xl"]
                sqh, sql = st["sqh"], st["sql"]
                i0 = i * 128
                cand = spool.tile([C, 72], F32, tag="cand")
                V32 = wpool.tile([C, N], F32, tag="V32", bufs=2, name="V32")
                for k, (c0, csz) in enumerate(CHUNKS):
                    Vp = chpool.tile([C, csz], F32, tag="ch", name="V")
                    nc.tensor.matmul(Vp[:], Xh[:, i0:i0 + 128],
                                     Xh[:, c0:c0 + csz],
                                     start=True, stop=False,
                                     skip_group_check=True)
                    nc.tensor.matmul(Vp[:], Xh[:, i0:i0 + 128],
                                     Xl[:, c0:c0 + csz],
                                     start=False, stop=False,
                                     skip_group_check=True)
                    nc.tensor.matmul(Vp[:], Xl[:, i0:i0 + 128],
                                     Xh[:, c0:c0 + csz],
                                     start=False, stop=False,
                                     skip_group_check=True)
                    p = (k % 4) * 32
                    nc.tensor.matmul(Vp[:], ones16[p:p + 1, 0:C],
                                     sqh[p:p + 1, c0:c0 + csz],
                                     start=False, stop=False,
                                     skip_group_check=True,
                                     tile_position=(p, 0))
                    nc.tensor.matmul(Vp[:], ones16[p:p + 1, 0:C],
                                     sql[p:p + 1, c0:c0 + csz],
                                     start=False, stop=True,
                                     skip_group_check=True,
                                     tile_position=(p, 0))
                    # stash V chunk to SBUF (bit-exact), freeing PSUM;
                    # threshold AND mask both read the stash
                    nc.scalar.copy(V32[:, c0:c0 + csz], Vp[:])
                    for s in range(csz // SEG):
                        g = 2 * k + s
                        nc.vector.max(cand[:, g * 8:(g + 1) * 8],
                                      V32[:, c0 + s * SEG:c0 + (s + 1) * SEG])
                top8 = spool.tile([C, 8], F32, tag="top8")
                nc.vector.max(top8[:], cand[:])
                cand2 = spool.tile([C, 72], F32, tag="cand2")
                nc.vector.match_replace(cand2[:], top8[:], cand[:], -1e30)
                next8 = spool.tile([C, 8], F32, tag="next8")
                nc.vector.max(next8[:], cand2[:])
                vv = spool.tile([C, 1], F32, tag="vv")
                nc.vector.tensor_add(vv[:], next8[:, 0:1], next8[:, 1:2])
                tneg = spool.tile([C, 1], F32, tag="tneg")
                nc.vector.tensor_scalar_mul(tneg[:], vv[:], -0.5)
                # previous block's mask goes out now (keeps ACT queue flowing)
                flush_mask()
                pending.append((b, i0, V32, tneg))

            # ------- phase B: NS accumulation via xbar-transposed mask -----
            def phase_b_block(b, j, dma=None):
                st = S[b]
                j0 = j * 128
                MT = mpool.tile([C, N], FP16, tag="MTT")
                eng = dma if dma is not None else nc.sync
                eng.dma_start_transpose(MT[:], st["Mdram"][:, j0:j0 + 128])
                for k, (c0, csz) in enumerate(CHUNKS):
                    nc.tensor.matmul(st["ns"][k][:],
                                     st["XTb"][:, j0:j0 + 128],
                                     MT[:, c0:c0 + csz],
                                     start=(j == 0), stop=(j == NB - 1),
                                     skip_group_check=True)

            def alloc_ns(b):
                S[b]["ns"] = [npool.tile([C, csz], F32, tag=f"ns{k}",
                                         name=f"ns{k}")
                              for k, (c0, csz) in enumerate(CHUNKS)]

            def emit_ot(b):
                st = S[b]
                h16 = wpool.tile([C, N], FP16, tag="h16", bufs=1,
                                 name=f"h16_{b}")
                OT_sb = kpool.tile([C, N], F32, tag="OT", bufs=BPC,
                                   name=f"OT{b}")
                st["OT_sb"] = OT_sb
                sqsc = wpool.tile([C, 512], F32, tag="sqsc", bufs=2,
                                  name=f"qs{b}")
                for k, (c0, csz) in enumerate(CHUNKS):
                    # h2(fp16) = 0.025 * ns + (0.05*X0 + 0.025*total)
                    nc.vector.scalar_tensor_tensor(h16[:, c0:c0 + csz],
                                                   st["ns"][k][:], 0.025,
                                                   st["X0"][:, c0:c0 + csz],
                                                   op0=ALU.mult, op1=ALU.add)
                    OT = chpool.tile([C, csz], F32, tag="ch", name="OT")
                    nc.tensor.matmul(OT[:], wT16[:], h16[:, c0:c0 + csz],
                                     start=True, stop=False,
                                     skip_group_check=True)
                    nc.tensor.matmul(OT[:], eye16[:], h16[:, c0:c0 + csz],
                                     start=False, stop=False,
                                     skip_group_check=True)
                    nc.tensor.matmul(OT[:], halfb16[:], ones_r16[0:1, 0:csz],
                                     start=False, stop=True,
                                     skip_group_check=True)
                    col = b * 5 + k
                    nc.scalar.activation(OT_sb[:, c0:c0 + csz], OT[:], AF.Copy,
                                         accum_out=s1all[:, col:col + 1])
                    nc.scalar.activation(sqsc[:, 0:csz], OT[:], AF.Square,
                                         accum_out=s2all[:, col:col + 1])

            # batch 0 phase A
            for i in range(NB):
                phase_a_block(0, i)
            flush_mask()
            # batch 1 phase A interleaved with batch 0 phase B
            alloc_ns(0)
            for j in range(NB):
                phase_a_block(1, j)
                phase_b_block(0, j)
            flush_mask()
            emit_ot(0)
            # batch 1 phase B; residual X reload for finalize (overlapped)
            alloc_ns(1)
            for b in range(BPC):
                Xfin = wpool.tile([C, N], F32, tag="V32", bufs=2,
                                  name=f"Xfin{b}")
                nc.sync.dma_start(Xfin[:],
                                  x_d[b].rearrange("c h w -> c (h w)"))
                S[b]["Xfin"] = Xfin
            for j in range(NB):
                phase_b_block(1, j)
            emit_ot(1)

            # ---------------- BN stats all-gather + local reduce ----------
            S12 = cpool.tile([C, 2], F32)
            nc.vector.reduce_sum(S12[:, 0:1], s1all[:],
                                 axis=mybir.AxisListType.X)
            nc.vector.reduce_sum(S12[:, 1:2], s2all[:],
                                 axis=mybir.AxisListType.X)
            in_b = dpool.tile([C, 2], F32, tag="arin")
            out_b = dpool.tile([C, 2], F32, tag="arout")
            nc.sync.dma_start(in_b[:], S12[:])
            nc.gpsimd.collective_compute(
                "AllReduce", ALU.add,
                replica_groups=[list(range(N_CORES))],
                ins=[in_b.opt()], outs=[out_b.opt()])
            g12 = cpool.tile([C, 2], F32)
            nc.sync.dma_start(g12[:], out_b[:])

            mean = cpool.tile([C, 1], F32)
            nc.vector.tensor_scalar_mul(mean[:], g12[:, 0:1], 1.0 / CNT)
            ex2 = cpool.tile([C, 1], F32)
            nc.vector.tensor_scalar_mul(ex2[:], g12[:, 1:2], 1.0 / CNT)
            m2 = cpool.tile([C, 1], F32)
            nc.vector.tensor_mul(m2[:], mean[:], mean[:])
            var = cpool.tile([C, 1], F32)
            nc.vector.tensor_sub(var[:], ex2[:], m2[:])
            vpe = cpool.tile([C, 1], F32)
            nc.vector.tensor_scalar_add(vpe[:], var[:], EPS)
            std = cpool.tile([C, 1], F32)
            nc.scalar.sqrt(std[:], vpe[:])
            inv = cpool.tile([C, 1], F32)
            nc.vector.reciprocal(inv[:], std[:])
            scale = cpool.tile([C, 1], F32)
            nc.vector.tensor_mul(scale[:], gcol[:], inv[:])
            ms = cpool.tile([C, 1], F32)
            nc.vector.tensor_mul(ms[:], mean[:], scale[:])
            shift = cpool.tile([C, 1], F32)
            nc.vector.tensor_sub(shift[:], bcol[:], ms[:])

            # ---------------- finalize ----------------
            for b in range(BPC):
                st = S[b]
                t3 = wpool.tile([C, N], F32, tag="fin", bufs=2, name="t3")
                nc.vector.scalar_tensor_tensor(t3[:], st["OT_sb"][:],
                                               scale[:, 0:1], st["Xfin"][:],
                                               op0=ALU.mult, op1=ALU.add)
                y = wpool.tile([C, N], F32, tag="fin", bufs=2, name="y")
                nc.scalar.activation(y[:], t3[:], AF.Relu,
                                     bias=shift[:, 0:1])
                nc.sync.dma_start(out_d[b].rearrange("c h w -> c (h w)"), y[:])

    nc.compile()
    return nc


def _get_nc():
    if "nc" not in _cache:
        _cache["nc"] = _build()
    return _cache["nc"]


def kernel(**inputs):
    x = np.ascontiguousarray(inputs["x"], dtype=np.float32)
    x0 = np.ascontiguousarray(inputs["x_0"], dtype=np.float32)
    w_lin = np.ascontiguousarray(inputs["W_lin"], dtype=np.float32)
    b_lin = np.ascontiguousarray(inputs["b_lin"], dtype=np.float32)
    gamma = np.ascontiguousarray(inputs["gamma"], dtype=np.float32)
    beta = np.ascontiguousarray(inputs["beta_bn"], dtype=np.float32)

    nc = _get_nc()
    wT = np.ascontiguousarray(w_lin.T)
    brow = b_lin.reshape(1, C)
    gcol = gamma.reshape(C, 1)
    bcol = beta.reshape(C, 1)
    eye = np.eye(C, dtype=np.float32)

    in_maps = []
    for i in range(N_CORES):
        in_maps.append({
            "x": np.ascontiguousarray(x[i * BPC:(i + 1) * BPC]),
            "x0": np.ascontiguousarray(x0[i * BPC:(i + 1) * BPC]),
            "wT": wT, "brow": brow, "gcol": gcol, "bcol": bcol, "eye": eye,
        })

    res = run_bass_kernel_spmd(nc, in_maps, list(range(N_CORES)))
    _cache["exec_time_ns"] = res.exec_time_ns
    out = np.concatenate([res.results[i]["out"] for i in range(N_CORES)],
                         axis=0)
    return out.astype(np.float32)
